# revision 22
# baseline (speedup 1.0000x reference)
"""Trainium2 Bass kernel for nn_EnsemblePolicyHeads (MoE routing head).

Self-contained: accepts FULL inputs, shards batch across the 8 NeuronCores
(data parallel, weights replicated), returns the FULL [8192, 64] output.

v3: host-prepared fp16 layouts (no on-device transposes/casts); attn row
broadcast on GpSimd (partition_broadcast) instead of PE matmuls; DMA issue
order tuned so the PE pipeline starts as early as possible.
"""
import sys

for _p in ("/opt/trn_rl_repo",):
    if _p not in sys.path:
        sys.path.insert(0, _p)


import numpy as np
from contextlib import ExitStack

import concourse.bass as bass
import concourse.tile as tile
from concourse import bacc, mybir
from concourse.tile_rust import add_dep_helper

F32 = mybir.dt.float32
F16 = mybir.dt.float16
F8 = mybir.dt.float8e4
AF = mybir.ActivationFunctionType
ALU = mybir.AluOpType

D = 2048      # input dim
H = 128       # hidden
O = 64        # output dim
E = 16        # num experts
P = 128
KO = D // P   # 16 k-slices
NT_SIZE = 512
N_CORES = 8
B_TOTAL = 8192
BC = B_TOTAL // N_CORES
NT = BC // NT_SIZE
SUBS = NT_SIZE // P   # 128-row blocks per nt
ZCHUNK = 4            # ko's per z DMA chunk
KO16 = 12             # leading k-slices in fp16
KO8 = 2               # trailing 256-wide fp8 DoubleRow k-blocks
SZ = 32.0             # fp8 z scale
SW = 8192.0           # fp8 W1 scale
SF = 512.0            # fp16 operand scale (2**9); products match fp8's 2**18
DR = None             # set in build_kernel


def build_kernel():
    nc = bacc.Bacc("TRN2", target_bir_lowering=False, debug=False)
    # pre-transposed inputs (host-prepared)
    zT_ap = nc.dram_tensor("zT", [P, NT, KO16, NT_SIZE], F16, kind="ExternalInput").ap()
    ztl_ap = nc.dram_tensor("ztl", [P, NT, KO8 * 2, NT_SIZE], F16, kind="ExternalInput").ap()
    z8_ap = nc.dram_tensor("z8", [P, NT, KO8, 2, NT_SIZE], F8, kind="ExternalInput").ap()
    W1T_ap = nc.dram_tensor("W1T", [P, E, KO16, H], F16, kind="ExternalInput").ap()
    W18_ap = nc.dram_tensor("W18", [P, E, KO8, 2, H], F8, kind="ExternalInput").ap()
    WaT_ap = nc.dram_tensor("WaT", [P, KO, P], F16, kind="ExternalInput").ap()
    W2T_ap = nc.dram_tensor("W2T", [P, E, P], F16, kind="ExternalInput").ap()
    b1T_ap = nc.dram_tensor("b1T", [P, E], F32, kind="ExternalInput").ap()
    b2_ap = nc.dram_tensor("b2", [E, P], F16, kind="ExternalInput").ap()
    ba_ap = nc.dram_tensor("ba", [E, 1], F32, kind="ExternalInput").ap()
    idm_ap = nc.dram_tensor("idm", [P, P], F32, kind="ExternalInput").ap()
    id16_ap = nc.dram_tensor("id16", [E, E], F16, kind="ExternalInput").ap()
    out_ap = nc.dram_tensor("out", [BC, O], F32, kind="ExternalOutput").ap()

    with tile.TileContext(nc) as tc, ExitStack() as ctx:
        persist = ctx.enter_context(tc.tile_pool(name="persist", bufs=1))
        t_pool = ctx.enter_context(tc.tile_pool(name="t", bufs=4))
        hm_pool = ctx.enter_context(tc.tile_pool(name="hm", bufs=4))
        eb_pool = ctx.enter_context(tc.tile_pool(name="eb", bufs=5))
        erow_pool = ctx.enter_context(tc.tile_pool(name="erow", bufs=5))
        res_pool = ctx.enter_context(tc.tile_pool(name="res", bufs=3))
        outsb_pool = ctx.enter_context(tc.tile_pool(name="outsb", bufs=3))
        psA = ctx.enter_context(tc.tile_pool(name="psA", bufs=3, space="PSUM"))
        psB = ctx.enter_context(tc.tile_pool(name="psB", bufs=1, space="PSUM"))
        psC = ctx.enter_context(tc.tile_pool(name="psC", bufs=2, space="PSUM"))
        psD = ctx.enter_context(tc.tile_pool(name="psD", bufs=2, space="PSUM"))

        # ---- persistent tiles ----
        zT = persist.tile([P, NT, KO16, NT_SIZE], F16)
        ztl = persist.tile([P, NT, KO8 * 2, NT_SIZE], F16)
        z8T = persist.tile([P, NT, KO8, 2, NT_SIZE], F8)
        W1T = persist.tile([P, E, KO16, H], F16)
        W18T = persist.tile([P, E, KO8, 2, H], F8)
        WaT = persist.tile([P, KO, P], F16)
        W2T = persist.tile([P, E, P], F16)
        b1T = persist.tile([P, E], F32)
        b2sb = persist.tile([E, P], F16)
        ba_sb = persist.tile([E, 1], F32)
        idm = persist.tile([P, P], F32)
        id16 = persist.tile([E, E], F16)
        expT = persist.tile([E, BC], F16)
        attn_be = persist.tile([P, BC // P, E], F32)
        denomT = persist.tile([P, BC // P], F32)
        recipT = persist.tile([P, BC // P], F32)

        # ---- loads. sync ring: startup-critical tensors in consumption
        # order, interleaved at fine granularity so the PE can start on the
        # first k-chunks. gpsimd ring (SWDGE): the big W1T tail, ungated
        # (gating via deps would block that sequencer's later compute ops).
        # scalar ring: small early tiles only.
        # PE warm-up: dummy matmuls on a zeroed scratch tile while z lands —
        # ramps the PE clock out of its low p-state before real work starts.
        scratch = persist.tile([P, NT_SIZE], F16)
        nc.gpsimd.memset(scratch, 0.0)
        for _ in range(16):
            ps_w = psB.tile([P, NT_SIZE], F32, tag="ps_l")
            nc.tensor.matmul(ps_w[:], scratch[:, :P], scratch[:],
                             start=True, stop=True)

        for c0 in range(0, KO16, ZCHUNK):
            nc.sync.dma_start(
                zT[:, 0, c0:c0 + ZCHUNK, :], zT_ap[:, 0, c0:c0 + ZCHUNK, :])
        nc.sync.dma_start(ztl[:, 0], ztl_ap[:, 0])
        z0_last = nc.sync.dma_start(z8T[:, 0], z8_ap[:, 0])
        nc.sync.dma_start(WaT[:], WaT_ap[:])
        nc.sync.dma_start(ba_sb[:], ba_ap[:])
        nc.sync.dma_start(W1T[:, 0], W1T_ap[:, 0])
        nc.sync.dma_start(W1T[:, 1], W1T_ap[:, 1])
        # z-nt1 held behind z-nt0 so nt0 (startup-critical) gets the
        # bandwidth; nt1 is not consumed until much later anyway.
        for nt in range(1, NT):
            for c0 in range(0, KO16, ZCHUNK):
                zd = nc.sync.dma_start(
                    zT[:, nt, c0:c0 + ZCHUNK, :], zT_ap[:, nt, c0:c0 + ZCHUNK, :])
                if c0 == 0:
                    add_dep_helper(z0_last.ins, zd.ins, reason="nt0 z first")
            nc.sync.dma_start(ztl[:, nt], ztl_ap[:, nt])
            nc.sync.dma_start(z8T[:, nt], z8_ap[:, nt])
        nc.scalar.dma_start(id16[:], id16_ap[:])
        nc.scalar.dma_start(b1T[:], b1T_ap[:])
        nc.scalar.dma_start(b2sb[:], b2_ap[:])
        nc.scalar.dma_start(W2T[:], W2T_ap[:])
        # big W1T tail on the gpsimd ring, held behind the nt0 z loads so z
        # gets full HBM bandwidth at startup (blocking the gpsimd sequencer
        # is safe: its next ops are the broadcasts, which need exp(nt0) and
        # hence all of z-nt0 anyway).
        nc.gpsimd.dma_start(W1T[:, 2], W1T_ap[:, 2])
        nc.gpsimd.dma_start(W1T[:, 3], W1T_ap[:, 3])
        nc.gpsimd.dma_start(W18T[:], W18_ap[:])

        def logits_nt(nt):
            bs = slice(nt * NT_SIZE, (nt + 1) * NT_SIZE)
            ps_l = psB.tile([P, NT_SIZE], F32, tag="ps_l")
            for ko in range(KO16):
                nc.tensor.matmul(
                    ps_l[:], WaT[:, ko, :], zT[:, nt, ko, :],
                    start=(ko == 0), stop=False)
            for c in range(KO8 * 2):
                nc.tensor.matmul(
                    ps_l[:], WaT[:, KO16 + c, :], ztl[:, nt, c, :],
                    start=False, stop=(c == KO8 * 2 - 1))
            nc.scalar.activation(expT[:, bs], ps_l[:E, :], AF.Exp,
                                 bias=ba_sb[:], scale=1.0 / SF)

        def denom_nt(nt):
            for sub in range(SUBS):
                blk = nt * SUBS + sub
                ps_t = psD.tile([P, E], F16, tag="ps_tr")
                nc.tensor.transpose(
                    ps_t[:], expT[:, blk * P:(blk + 1) * P], id16[:])
                nc.scalar.copy(attn_be[:, blk, :], ps_t[:])
            nts = slice(nt * SUBS, (nt + 1) * SUBS)
            nc.vector.reduce_sum(
                denomT[:, nts, None], attn_be[:, nts, :], axis=mybir.AxisListType.X)
            nc.vector.reciprocal(recipT[:, nts], denomT[:, nts])

        def finalize_nt(nt, ps_o):
            for sub in range(SUBS):
                blk = nt * SUBS + sub
                res = res_pool.tile([O, P], F32)
                nc.scalar.copy(res[:], ps_o[:O, sub * P:(sub + 1) * P])
                ps_t2 = psD.tile([P, O], F32, tag="ps_tr")
                nc.tensor.transpose(ps_t2[:], res[:], idm[:O, :O])
                outsb = outsb_pool.tile([P, O], F32)
                nc.scalar.activation(outsb[:], ps_t2[:], AF.Copy,
                                     scale=recipT[:, blk:blk + 1])
                nc.sync.dma_start(out_ap[blk * P:(blk + 1) * P, :], outsb[:])

        # ---- main loop, software-pipelined ----
        logits_nt(0)
        denom_nt(0)

        pend_w2 = []      # deque of (e, hm, ps_o) deferred W2 matmuls
        pend_fin = None   # (nt, ps_o) to finalize after next mm1 group

        def flush_w2(keep):
            while len(pend_w2) > keep:
                pe, phm, po = pend_w2.pop(0)
                nc.tensor.matmul(po[:], W2T[:, pe, :], phm[:],
                                 start=False, stop=(pe == E - 1))

        for nt in range(NT):
            bs = slice(nt * NT_SIZE, (nt + 1) * NT_SIZE)
            ps_o = psC.tile([P, NT_SIZE], F32)
            # b2 contribution first: only needs expT, keeps the tail short
            nc.tensor.matmul(ps_o[:], b2sb[:], expT[:, bs],
                             start=True, stop=False)
            for e in range(E):
                erow = erow_pool.tile([1, NT_SIZE], F16)
                nc.sync.dma_start(erow[:], expT[e:e + 1, bs])
                if nt == 0 and e + 4 < E:
                    nc.gpsimd.dma_start(W1T[:, e + 4], W1T_ap[:, e + 4])
                if nt == 0 and e == E - 4:
                    nc.gpsimd.dma_start(idm[:], idm_ap[:])
                ebc = eb_pool.tile([P, NT_SIZE], F16)
                nc.gpsimd.partition_broadcast(ebc[:], erow[:])
                ps_h = psA.tile([P, NT_SIZE], F32)
                for ko in range(KO16):
                    nc.tensor.matmul(
                        ps_h[:], W1T[:, e, ko, :], zT[:, nt, ko, :],
                        start=(ko == 0), stop=False)
                for c in range(KO8):
                    nc.tensor.matmul(
                        ps_h[:], W18T[:, e, c, :, :], z8T[:, nt, c, :, :],
                        start=False, stop=(c == KO8 - 1),
                        perf_mode=mybir.MatmulPerfMode.DoubleRow)
                if pend_fin is not None and e == 0:
                    finalize_nt(*pend_fin)
                    pend_fin = None
                flush_w2(2 if e < E - 1 else 0)
                if e == 13 and nt + 1 < NT:
                    logits_nt(nt + 1)
                if e == 14 and nt + 1 < NT:
                    denom_nt(nt + 1)
                t = t_pool.tile([P, NT_SIZE], F16)
                nc.scalar.activation(t[:], ps_h[:], AF.Relu, bias=b1T[:, e:e + 1],
                                     scale=1.0 / (SZ * SW))
                hm = hm_pool.tile([P, NT_SIZE], F16)
                nc.vector.tensor_tensor(hm[:], t[:], ebc[:], ALU.mult)
                pend_w2.append((e, hm, ps_o))
            flush_w2(0)
            pend_fin = (nt, ps_o)
        finalize_nt(*pend_fin)

    nc.compile()
    return nc


_nc_cache = {}


def _get_nc():
    if "nc" not in _nc_cache:
        _nc_cache["nc"] = build_kernel()
    return _nc_cache["nc"]


def prep_inputs(z_i, W1, b1, W2, b2, Wa, ba):
    """Host-side layout prep shared by all cores (weights) + per-core z."""
    import ml_dtypes
    E4M3 = ml_dtypes.float8_e4m3fn
    z = np.asarray(z_i, dtype=np.float32).reshape(B_TOTAL, D)
    # zt[core][ki, nt, ko, b] = z[core*BC + nt*512 + b, 128*ko + ki]
    zt = z.reshape(N_CORES, NT, NT_SIZE, KO, P).transpose(0, 4, 1, 3, 2)
    z16 = (zt[:, :, :, :KO16, :] * SF).astype(np.float16)
    ztl = (zt[:, :, :, KO16:, :] * SF).astype(np.float16)
    z8 = (zt[:, :, :, KO16:, :] * SZ).astype(E4M3).reshape(
        N_CORES, P, NT, KO8, 2, NT_SIZE)
    z_cores = [(np.ascontiguousarray(z16[c]), np.ascontiguousarray(ztl[c]),
                np.ascontiguousarray(z8[c])) for c in range(N_CORES)]

    w1t = np.asarray(W1, np.float32).reshape(E, KO, P, H).transpose(2, 0, 1, 3)
    W1T = np.ascontiguousarray(w1t[:, :, :KO16] * SF).astype(np.float16)
    W18 = np.ascontiguousarray(
        (w1t[:, :, KO16:] * SW).astype(E4M3).reshape(P, E, KO8, 2, H))
    WaT = np.zeros((P, KO, P), np.float16)
    WaT[:, :, :E] = np.asarray(Wa, np.float32).reshape(KO, P, E).transpose(1, 0, 2)
    W2T = np.zeros((P, E, P), np.float16)
    W2T[:, :, :O] = np.asarray(W2, np.float32).transpose(1, 0, 2)
    b1T = np.ascontiguousarray(np.asarray(b1, np.float32).T)
    b2h = np.zeros((E, P), np.float16)
    b2h[:, :O] = np.asarray(b2, np.float32)
    bav = np.asarray(ba, np.float32).reshape(E, 1)
    idm = np.eye(P, dtype=np.float32)
    id16 = np.eye(E, dtype=np.float16)
    shared = dict(W1T=W1T, W18=W18, WaT=WaT, W2T=W2T, b1T=b1T, b2=b2h,
                  ba=bav, idm=idm, id16=id16)
    return z_cores, shared


def kernel(z_i, W1, b1, W2, b2, Wa, ba):
    from concourse.bass_utils import run_bass_kernel_spmd

    z_cores, shared = prep_inputs(z_i, W1, b1, W2, b2, Wa, ba)
    nc = _get_nc()
    in_maps = [dict(zT=z_cores[c][0], ztl=z_cores[c][1], z8=z_cores[c][2],
                    **shared) for c in range(N_CORES)]
    res = run_bass_kernel_spmd(nc, in_maps, core_ids=list(range(N_CORES)))
    return np.concatenate([res.results[c]["out"] for c in range(N_CORES)], axis=0)


# revision 23
# speedup vs baseline: 1.1135x; 1.1135x over previous
"""Trainium2 Bass kernel for nn_EnsemblePolicyHeads (MoE routing head).

Self-contained: accepts FULL inputs, shards batch across the 8 NeuronCores
(data parallel, weights replicated), returns the FULL [8192, 64] output.

Design: host-prepared fp16 layouts (no on-device transposes/casts); all
matmul stationaries padded to 128 columns (one PE tile config, no
reconfig); attn row broadcast on GpSimd; PE warm-up matmuls ramp the clock
while z lands; DMA rings ordered so z-nt0 gets bandwidth first.
Note: fp8 DoubleRow was tried and reverted — any fp8 use downclocks the
whole core 2.4->2.0 GHz, a net loss at accuracy-viable fp8 fractions.
"""
import sys

for _p in ("/opt/trn_rl_repo",):
    if _p not in sys.path:
        sys.path.insert(0, _p)


import numpy as np
from contextlib import ExitStack

import concourse.bass as bass
import concourse.tile as tile
from concourse import bacc, mybir
from concourse.tile_rust import add_dep_helper

F32 = mybir.dt.float32
F16 = mybir.dt.float16
AF = mybir.ActivationFunctionType
ALU = mybir.AluOpType

D = 2048      # input dim
H = 128       # hidden
O = 64        # output dim
E = 16        # num experts
P = 128
KO = D // P   # 16 k-slices
NT_SIZE = 512
N_CORES = 8
B_TOTAL = 8192
BC = B_TOTAL // N_CORES
NT = BC // NT_SIZE
SUBS = NT_SIZE // P   # 128-row blocks per nt
ZCHUNK = 4            # ko's per z DMA chunk


def build_kernel():
    nc = bacc.Bacc("TRN2", target_bir_lowering=False, debug=False)
    # pre-transposed inputs (host-prepared)
    zT_ap = nc.dram_tensor("zT", [P, NT, KO, NT_SIZE], F16, kind="ExternalInput").ap()
    W1T_ap = nc.dram_tensor("W1T", [P, E, KO, H], F16, kind="ExternalInput").ap()
    WaT_ap = nc.dram_tensor("WaT", [P, KO, P], F16, kind="ExternalInput").ap()
    W2T_ap = nc.dram_tensor("W2T", [P, E, P], F16, kind="ExternalInput").ap()
    b1T_ap = nc.dram_tensor("b1T", [P, E], F32, kind="ExternalInput").ap()
    b2_ap = nc.dram_tensor("b2", [E, P], F16, kind="ExternalInput").ap()
    ba_ap = nc.dram_tensor("ba", [E, 1], F32, kind="ExternalInput").ap()
    idm_ap = nc.dram_tensor("idm", [P, P], F32, kind="ExternalInput").ap()
    id16_ap = nc.dram_tensor("id16", [E, E], F16, kind="ExternalInput").ap()
    out_ap = nc.dram_tensor("out", [BC, O], F32, kind="ExternalOutput").ap()

    with tile.TileContext(nc) as tc, ExitStack() as ctx:
        persist = ctx.enter_context(tc.tile_pool(name="persist", bufs=1))
        t_pool = ctx.enter_context(tc.tile_pool(name="t", bufs=4))
        hm_pool = ctx.enter_context(tc.tile_pool(name="hm", bufs=4))
        eb_pool = ctx.enter_context(tc.tile_pool(name="eb", bufs=5))
        erow_pool = ctx.enter_context(tc.tile_pool(name="erow", bufs=5))
        res_pool = ctx.enter_context(tc.tile_pool(name="res", bufs=3))
        outsb_pool = ctx.enter_context(tc.tile_pool(name="outsb", bufs=3))
        psA = ctx.enter_context(tc.tile_pool(name="psA", bufs=3, space="PSUM"))
        psB = ctx.enter_context(tc.tile_pool(name="psB", bufs=1, space="PSUM"))
        psC = ctx.enter_context(tc.tile_pool(name="psC", bufs=2, space="PSUM"))
        psD = ctx.enter_context(tc.tile_pool(name="psD", bufs=2, space="PSUM"))

        # ---- persistent tiles ----
        zT = persist.tile([P, NT, KO, NT_SIZE], F16)
        W1T = persist.tile([P, E, KO, H], F16)
        WaT = persist.tile([P, KO, P], F16)
        W2T = persist.tile([P, E, P], F16)
        b1T = persist.tile([P, E], F32)
        b2sb = persist.tile([E, P], F16)
        ba_sb = persist.tile([E, 1], F32)
        idm = persist.tile([P, P], F32)
        id16 = persist.tile([E, E], F16)
        expT = persist.tile([E, BC], F16)
        attn_be = persist.tile([P, BC // P, E], F32)
        denomT = persist.tile([P, BC // P], F32)
        recipT = persist.tile([P, BC // P], F32)

        # PE warm-up: dummy matmuls on a zeroed scratch tile while z lands —
        # ramps the PE clock out of its low p-state before real work starts.
        scratch = persist.tile([P, NT_SIZE], F16)
        nc.gpsimd.memset(scratch, 0.0)
        for _ in range(16):
            ps_w = psB.tile([P, NT_SIZE], F32, tag="ps_l")
            nc.tensor.matmul(ps_w[:], scratch[:, :P], scratch[:],
                             start=True, stop=True)

        # ---- loads. sync ring: startup-critical tensors in consumption
        # order. gpsimd ring (SWDGE): the big W1T tail. scalar ring: small
        # early tiles.
        for c0 in range(0, KO, ZCHUNK):
            z0_last = nc.sync.dma_start(
                zT[:, 0, c0:c0 + ZCHUNK, :], zT_ap[:, 0, c0:c0 + ZCHUNK, :])
        nc.sync.dma_start(WaT[:], WaT_ap[:])
        nc.sync.dma_start(ba_sb[:], ba_ap[:])
        nc.sync.dma_start(W1T[:, 0], W1T_ap[:, 0])
        nc.sync.dma_start(W1T[:, 1], W1T_ap[:, 1])
        # z-nt1 held behind z-nt0 so nt0 (startup-critical) gets bandwidth.
        for nt in range(1, NT):
            for c0 in range(0, KO, ZCHUNK):
                zd = nc.sync.dma_start(
                    zT[:, nt, c0:c0 + ZCHUNK, :], zT_ap[:, nt, c0:c0 + ZCHUNK, :])
                if c0 == 0:
                    add_dep_helper(z0_last.ins, zd.ins, reason="nt0 z first")
        nc.scalar.dma_start(id16[:], id16_ap[:])
        nc.scalar.dma_start(b1T[:], b1T_ap[:])
        nc.scalar.dma_start(b2sb[:], b2_ap[:])
        nc.scalar.dma_start(W2T[:], W2T_ap[:])
        nc.gpsimd.dma_start(W1T[:, 2], W1T_ap[:, 2])
        nc.gpsimd.dma_start(W1T[:, 3], W1T_ap[:, 3])

        def logits_nt(nt):
            bs = slice(nt * NT_SIZE, (nt + 1) * NT_SIZE)
            ps_l = psB.tile([P, NT_SIZE], F32, tag="ps_l")
            for ko in range(KO):
                nc.tensor.matmul(
                    ps_l[:], WaT[:, ko, :], zT[:, nt, ko, :],
                    start=(ko == 0), stop=(ko == KO - 1))
            nc.scalar.activation(expT[:, bs], ps_l[:E, :], AF.Exp, bias=ba_sb[:])

        def denom_nt(nt):
            for sub in range(SUBS):
                blk = nt * SUBS + sub
                ps_t = psD.tile([P, E], F16, tag="ps_tr")
                nc.tensor.transpose(
                    ps_t[:], expT[:, blk * P:(blk + 1) * P], id16[:])
                nc.scalar.copy(attn_be[:, blk, :], ps_t[:])
            nts = slice(nt * SUBS, (nt + 1) * SUBS)
            nc.vector.reduce_sum(
                denomT[:, nts, None], attn_be[:, nts, :], axis=mybir.AxisListType.X)
            nc.vector.reciprocal(recipT[:, nts], denomT[:, nts])

        def finalize_nt(nt, ps_o):
            for sub in range(SUBS):
                blk = nt * SUBS + sub
                res = res_pool.tile([O, P], F32)
                nc.scalar.copy(res[:], ps_o[:O, sub * P:(sub + 1) * P])
                ps_t2 = psD.tile([P, O], F32, tag="ps_tr")
                nc.tensor.transpose(ps_t2[:], res[:], idm[:O, :O])
                outsb = outsb_pool.tile([P, O], F32)
                nc.scalar.activation(outsb[:], ps_t2[:], AF.Copy,
                                     scale=recipT[:, blk:blk + 1])
                nc.sync.dma_start(out_ap[blk * P:(blk + 1) * P, :], outsb[:])

        # ---- main loop, software-pipelined ----
        logits_nt(0)
        denom_nt(0)

        pend_w2 = []      # deque of (e, hm, ps_o) deferred W2 matmuls
        pend_fin = None   # (nt, ps_o) to finalize after next mm1 group

        def flush_w2(keep):
            while len(pend_w2) > keep:
                pe, phm, po = pend_w2.pop(0)
                nc.tensor.matmul(po[:], W2T[:, pe, :], phm[:],
                                 start=False, stop=(pe == E - 1))

        for nt in range(NT):
            bs = slice(nt * NT_SIZE, (nt + 1) * NT_SIZE)
            ps_o = psC.tile([P, NT_SIZE], F32)
            # b2 contribution first: only needs expT, keeps the tail short
            nc.tensor.matmul(ps_o[:], b2sb[:], expT[:, bs],
                             start=True, stop=False)
            for e in range(E):
                erow = erow_pool.tile([1, NT_SIZE], F16)
                nc.sync.dma_start(erow[:], expT[e:e + 1, bs])
                if nt == 0 and e + 4 < E:
                    nc.gpsimd.dma_start(W1T[:, e + 4], W1T_ap[:, e + 4])
                if nt == 0 and e == E - 4:
                    nc.gpsimd.dma_start(idm[:], idm_ap[:])
                ebc = eb_pool.tile([P, NT_SIZE], F16)
                nc.gpsimd.partition_broadcast(ebc[:], erow[:])
                ps_h = psA.tile([P, NT_SIZE], F32)
                for ko in range(KO):
                    nc.tensor.matmul(
                        ps_h[:], W1T[:, e, ko, :], zT[:, nt, ko, :],
                        start=(ko == 0), stop=(ko == KO - 1))
                if pend_fin is not None and e == 0:
                    finalize_nt(*pend_fin)
                    pend_fin = None
                flush_w2(2 if e < E - 1 else 0)
                if e == 13 and nt + 1 < NT:
                    logits_nt(nt + 1)
                if e == 14 and nt + 1 < NT:
                    denom_nt(nt + 1)
                t = t_pool.tile([P, NT_SIZE], F16)
                nc.scalar.activation(t[:], ps_h[:], AF.Relu, bias=b1T[:, e:e + 1])
                hm = hm_pool.tile([P, NT_SIZE], F16)
                nc.vector.tensor_tensor(hm[:], t[:], ebc[:], ALU.mult)
                pend_w2.append((e, hm, ps_o))
            flush_w2(0)
            pend_fin = (nt, ps_o)
        finalize_nt(*pend_fin)

    nc.compile()
    return nc


_nc_cache = {}


def _get_nc():
    if "nc" not in _nc_cache:
        _nc_cache["nc"] = build_kernel()
    return _nc_cache["nc"]


def prep_inputs(z_i, W1, b1, W2, b2, Wa, ba):
    """Host-side layout prep shared by all cores (weights) + per-core z."""
    z = np.asarray(z_i, dtype=np.float32).reshape(B_TOTAL, D).astype(np.float16)
    # zT[core][ki, nt, ko, b] = z[core*BC + nt*512 + b, 128*ko + ki]
    zt = z.reshape(N_CORES, NT, NT_SIZE, KO, P).transpose(0, 4, 1, 3, 2)
    z_cores = [np.ascontiguousarray(zt[c]) for c in range(N_CORES)]

    W1T = np.ascontiguousarray(
        np.asarray(W1, np.float32).reshape(E, KO, P, H).transpose(2, 0, 1, 3)
    ).astype(np.float16)
    WaT = np.zeros((P, KO, P), np.float16)
    WaT[:, :, :E] = np.asarray(Wa, np.float32).reshape(KO, P, E).transpose(1, 0, 2)
    W2T = np.zeros((P, E, P), np.float16)
    W2T[:, :, :O] = np.asarray(W2, np.float32).transpose(1, 0, 2)
    b1T = np.ascontiguousarray(np.asarray(b1, np.float32).T)
    b2h = np.zeros((E, P), np.float16)
    b2h[:, :O] = np.asarray(b2, np.float32)
    bav = np.asarray(ba, np.float32).reshape(E, 1)
    idm = np.eye(P, dtype=np.float32)
    id16 = np.eye(E, dtype=np.float16)
    shared = dict(W1T=W1T, WaT=WaT, W2T=W2T, b1T=b1T, b2=b2h, ba=bav,
                  idm=idm, id16=id16)
    return z_cores, shared


def kernel(z_i, W1, b1, W2, b2, Wa, ba):
    from concourse.bass_utils import run_bass_kernel_spmd

    z_cores, shared = prep_inputs(z_i, W1, b1, W2, b2, Wa, ba)
    nc = _get_nc()
    in_maps = [dict(zT=z_cores[c], **shared) for c in range(N_CORES)]
    res = run_bass_kernel_spmd(nc, in_maps, core_ids=list(range(N_CORES)))
    return np.concatenate([res.results[c]["out"] for c in range(N_CORES)], axis=0)
